# revision 7
# baseline (speedup 1.0000x reference)
"""BiGraphConv (GNN message passing) Trainium2 kernel, 8-core SPMD.

out = x_dst @ W_self.T + b_self + scatter_add_dst(w_e * x_src[src_e]) @ W_nei.T

Streaming blocked-SpMM formulation (no device-side gather):
  host relayouts the edge list into a dense column format — column c holds
  128 edges (sorted by dst); mt[:, c, :] carries those edges' x_src rows
  (fp8 e3m4) and pm[:, off_c:off_c+W_c] carries their weights (fp16)
  scattered to chunk-local dst positions. The device streams both tables
  and, per column, runs one matmul mt_c.T @ pm_c accumulating into a PSUM
  bank covering one chunk of dsts; the transform
  (W_nei @ agg + W_self @ x_dst + b) follows per chunk in fp16.

Sharding: dst nodes are dealt snake-wise by degree into 8 cores x 25 chunks
so every (core, chunk) bucket has near-equal edge count -> near-zero column
padding and a single common program for all cores. Output is unpermuted
on host.
"""
import sys
import numpy as np

for _p in ("/opt/trn_rl_repo", "/root/.axon_site/_ro/trn_rl_repo"):
    if _p not in sys.path:
        sys.path.insert(0, _p)

from contextlib import ExitStack

import ml_dtypes
import concourse.bass as bass
import concourse.tile as tile
from concourse import bacc, mybir
from concourse.bass_utils import run_bass_kernel_spmd

# problem constants (hardcoded per task contract)
N_SRC = 100000
N_DST = 100000
E = 1250000
F = 64            # feature dim (in == out == 64)
NC = 8            # cores
SHARD = N_DST // NC           # 12500 dst rows per core
TCH = 500                     # dsts per PSUM bank chunk (500 f32 = 2000B)
NCHK = 25                     # chunks per core
CSZ = [TCH] * NCHK
COFF = np.r_[0, np.cumsum(CSZ)]
P = 128                       # edges per column
MTW = 32                      # mt stream tile width (columns)
BG = 4                        # chunks per pm/xdt DMA batch
MT_FP8 = True                 # x_src rows in fp8 e3m4 (else fp16)
PM_FP8 = False                # edge weights stay fp16 (error margin)
XDT_FP8 = True                # x_dst in fp8 e3m4


def _host_prep(x_src, x_dst, edge_index_sd, edge_weight, W_nei, W_self, b_self):
    src = np.asarray(edge_index_sd[0], dtype=np.int64)
    dst = np.asarray(edge_index_sd[1], dtype=np.int64)
    ew = np.asarray(edge_weight, dtype=np.float32)
    x_dst = np.asarray(x_dst, dtype=np.float32)
    x_src = np.asarray(x_src, dtype=np.float32)

    # --- dst -> (core, chunk) snake deal by degree --------------------------
    # Bucket (c, k) holds CSZ[k] dsts; smaller (tail) buckets only join the
    # final rounds, so they get the lowest-degree dsts (fast pipeline drain)
    # while every bucket's edge total stays balanced across cores.
    deg = np.bincount(dst, minlength=N_DST)
    order = np.argsort(-deg, kind="stable")          # degree descending
    R = max(CSZ)
    # bucket ids b = k * NC + c, active in rounds [R - CSZ[k], R)
    bk = np.repeat(np.arange(NCHK), NC)              # bucket -> chunk
    bc = np.tile(np.arange(NC), NCHK)                # bucket -> core
    dstmap = np.empty((NC, SHARD), dtype=np.int64)
    ptr = 0
    for r in range(R):
        act = np.flatnonzero(r >= R - np.array(CSZ)[bk])
        if r % 2 == 1:
            act = act[::-1]
        take = order[ptr:ptr + act.size]
        ptr += act.size
        pos = COFF[bk[act]] + (r - (R - np.array(CSZ)[bk[act]]))
        dstmap[bc[act], pos] = take
    assert ptr == N_DST
    coreof = np.empty(N_DST, dtype=np.int64)
    localof = np.empty(N_DST, dtype=np.int64)
    for c in range(NC):
        coreof[dstmap[c]] = c
        localof[dstmap[c]] = np.arange(SHARD)

    ecore = coreof[dst]
    elocal = localof[dst]

    # --- per-core edge sort and per-chunk column counts ---------------------
    per_core_edges = []
    cnt_ck = np.zeros((NC, NCHK), dtype=np.int64)
    for c in range(NC):
        sel = np.flatnonzero(ecore == c)
        el = elocal[sel]
        o = np.argsort(el, kind="stable")
        sel = sel[o]
        el = el[o]
        per_core_edges.append((el, src[sel], ew[sel]))
        cnt_ck[c] = np.bincount(
            np.searchsorted(COFF, el, side='right') - 1,
            minlength=NCHK)

    ncols_k = (cnt_ck.max(axis=0) + P - 1) // P      # common cols per chunk
    colbase_k = np.zeros(NCHK + 1, dtype=np.int64)
    np.cumsum(ncols_k, out=colbase_k[1:])
    totcols = int(colbase_k[-1])

    # --- per-column dst window (off, W) common across cores -----------------
    dmin = np.full(totcols, 1 << 30, dtype=np.int64)
    dmax = np.full(totcols, -1, dtype=np.int64)
    ecol = []   # per core: global column id per edge
    eslot = []
    for c in range(NC):
        el, _, _ = per_core_edges[c]
        chunk = np.searchsorted(COFF, el, side='right') - 1
        cs = np.r_[0, np.cumsum(cnt_ck[c])][:-1]
        pos = np.arange(el.size, dtype=np.int64) - cs[chunk]
        col = colbase_k[chunk] + pos // P
        slot = pos % P
        ecol.append(col)
        eslot.append(slot)
        dl = el - COFF[chunk]                         # chunk-local dst id
        np.minimum.at(dmin, col, dl)
        np.maximum.at(dmax, col, dl)
    empty = dmax < 0
    dmin[empty] = 0
    dmax[empty] = 0
    W_col = dmax - dmin + 1                           # rhs width per column
    pmoff = np.zeros(totcols + 1, dtype=np.int64)
    np.cumsum(W_col, out=pmoff[1:])
    totpm = int(pmoff[-1])
    pmbase_k = pmoff[colbase_k]                       # [NCHK+1]

    ftype = np.float16
    mtype = ml_dtypes.float8_e3m4 if MT_FP8 else np.float16
    ptype = ml_dtypes.float8_e3m4 if PM_FP8 else np.float16
    xtype = ml_dtypes.float8_e3m4 if XDT_FP8 else np.float16
    x_mt = x_src.astype(mtype)

    per_core = []
    for c in range(NC):
        el, es, ws_ = per_core_edges[c]
        col = ecol[c]
        slot = eslot[c]
        chunk = np.searchsorted(COFF, el, side='right') - 1
        dl = el - COFF[chunk]
        # pm scatter
        pm = np.zeros((P, totpm), dtype=ptype)
        pm[slot, pmoff[col] + (dl - dmin[col])] = ws_.astype(ptype)
        # mt gather (pad slots point at row 0; their pm entries are 0)
        sidx = np.zeros((P, totcols), dtype=np.int64)
        sidx[slot, col] = es
        mt = np.ascontiguousarray(x_mt[sidx].reshape(P, totcols * F))
        xdt = np.empty((F + 1, SHARD), dtype=xtype)
        xdt[:F] = x_dst[dstmap[c]].T.astype(xtype)
        xdt[F] = 1.0
        per_core.append({"mt": mt, "pm": pm, "xdt": xdt})

    meta = {
        "ncols_k": ncols_k, "colbase_k": colbase_k, "totcols": totcols,
        "W_col": W_col, "pmoff": pmoff, "totpm": totpm,
        "pmbase_k": pmbase_k, "dmin": dmin,
    }
    wc = np.zeros((F + 1, 2 * F), dtype=ftype)
    wc[:F, :F] = np.asarray(W_nei, np.float32).T.astype(ftype)
    wc[:F, F:] = np.asarray(W_self, np.float32).T.astype(ftype)
    wc[F, F:] = np.asarray(b_self, np.float32).astype(ftype)
    common = {"wc": wc}
    return meta, per_core, common, dstmap


def _build_program(meta):
    colbase_k = meta["colbase_k"]
    totcols = meta["totcols"]
    W_col = meta["W_col"]
    pmoff = meta["pmoff"]
    totpm = meta["totpm"]
    pmbase_k = meta["pmbase_k"]
    dmin = meta["dmin"]

    nc = bacc.Bacc("TRN2", target_bir_lowering=False, debug=False,
                   enable_asserts=False, num_devices=NC)
    f32 = mybir.dt.float32
    DT = mybir.dt.float16
    MDT = mybir.dt.float8e3 if MT_FP8 else DT
    PDT = mybir.dt.float8e3 if PM_FP8 else DT
    XDTT = mybir.dt.float8e3 if XDT_FP8 else DT

    mt_t = nc.dram_tensor("mt", (P, totcols * F), MDT, kind="ExternalInput")
    pm_t = nc.dram_tensor("pm", (P, totpm), PDT, kind="ExternalInput")
    xdt_t = nc.dram_tensor("xdt", (F + 1, SHARD), XDTT, kind="ExternalInput")
    wc_t = nc.dram_tensor("wc", (F + 1, 2 * F), DT, kind="ExternalInput")
    out_t = nc.dram_tensor("outT", (F, SHARD), DT, kind="ExternalOutput")

    NB_ = (NCHK + BG - 1) // BG            # pm/xdt batches
    n_mt_tiles = (totcols + MTW - 1) // MTW
    pmw_b = [int(pmbase_k[min((b + 1) * BG, NCHK)] - pmbase_k[b * BG])
             for b in range(NB_)]
    pmw_max = max(pmw_b)
    xdw_b = [int(COFF[min((b + 1) * BG, NCHK)] - COFF[b * BG])
             for b in range(NB_)]
    xdw_max = max(xdw_b)
    # out write batches: final batch is the last chunk alone
    wbounds = [(0, 4), (4, 8), (8, 12), (12, 16), (16, 20), (20, 24), (24, 25)]
    wmax = max(int(COFF[e] - COFF[s]) for s, e in wbounds)

    with tile.TileContext(nc) as tc:
        with ExitStack() as ctx:
            const = ctx.enter_context(tc.tile_pool(name="const", bufs=1))
            mtp = ctx.enter_context(tc.tile_pool(name="mtp", bufs=8))
            pmp = ctx.enter_context(tc.tile_pool(name="pmp", bufs=3))
            xdp = ctx.enter_context(tc.tile_pool(name="xdp", bufs=2))
            aggp = ctx.enter_context(tc.tile_pool(name="aggp", bufs=4))
            osbp = ctx.enter_context(tc.tile_pool(name="osbp", bufs=2))
            psa = ctx.enter_context(tc.tile_pool(name="psa", bufs=3,
                                                 space="PSUM"))
            pso = ctx.enter_context(tc.tile_pool(name="pso", bufs=3,
                                                 space="PSUM"))

            wc_s = const.tile([F + 1, 2 * F], DT)
            nc.scalar.dma_start(wc_s[:], wc_t.ap())

            mt_tiles = [None] * n_mt_tiles
            pm_tiles = [None] * NB_
            xdt_tiles = [None] * NB_

            def need_mt(m):
                if mt_tiles[m] is None:
                    w = min(MTW, totcols - m * MTW)
                    t = mtp.tile([P, MTW * F], MDT, tag="mt", name="mts")
                    nc.sync.dma_start(
                        t[:, :w * F], mt_t.ap()[:, m * MTW * F:
                                                (m * MTW + w) * F])
                    mt_tiles[m] = t
                return mt_tiles[m]

            def need_batch(b):
                if pm_tiles[b] is None:
                    w = pmw_b[b]
                    t = pmp.tile([P, pmw_max], PDT, tag="pm", name="pms")
                    nc.gpsimd.dma_start(
                        t[:, :w], pm_t.ap()[:, int(pmbase_k[b * BG]):
                                            int(pmbase_k[b * BG]) + w])
                    pm_tiles[b] = t
                    kw = xdw_b[b]
                    xo = int(COFF[b * BG])
                    xt = xdp.tile([F + 1, xdw_max], XDTT, tag="xd",
                                  name="xds")
                    nc.gpsimd.dma_start(
                        xt[:, :kw], xdt_t.ap()[:, xo:xo + kw])
                    xdt_tiles[b] = xt
                return pm_tiles[b], xdt_tiles[b]

            wb = 0
            osb_s = None
            for k in range(NCHK):
                b = k // BG
                pm_s, xdt_s = need_batch(b)
                pmb = int(pmbase_k[b * BG])
                csz = CSZ[k]
                # aggregate this chunk's columns into a PSUM bank
                ps = psa.tile([F, TCH], f32, tag="ps", name="ps")
                nc.vector.memset(ps[:, :csz], 0.0)
                c0, c1 = int(colbase_k[k]), int(colbase_k[k + 1])
                for c in range(c0, c1):
                    m = c // MTW
                    mt_s = need_mt(m)
                    lo = (c - m * MTW) * F
                    po = int(pmoff[c]) - pmb
                    wcw = int(W_col[c])
                    off = int(dmin[c])
                    nc.tensor.matmul(
                        out=ps[:, off:off + wcw],
                        lhsT=mt_s[:, lo:lo + F],
                        rhs=pm_s[:, po:po + wcw],
                        start=False, stop=(c == c1 - 1))
                agg_s = aggp.tile([F, TCH], DT, tag="agg", name="agg")
                nc.scalar.copy(agg_s[:, :csz], ps[:, :csz])
                # transform: W_self @ x_dst + bias (start), + W_nei @ agg
                kb = int(COFF[k] - COFF[b * BG])
                ps2 = pso.tile([F, TCH], f32, tag="ps2", name="ps2")
                nc.tensor.matmul(out=ps2[:, :csz], lhsT=wc_s[:F + 1, F:],
                                 rhs=xdt_s[:, kb:kb + csz],
                                 start=True, stop=False)
                nc.tensor.matmul(out=ps2[:, :csz], lhsT=wc_s[:F, :F],
                                 rhs=agg_s[:, :csz], start=False, stop=True)
                if k == wbounds[wb][0]:
                    osb_s = osbp.tile([F, wmax], DT, tag="osb",
                                      name="osb")
                ko = int(COFF[k] - COFF[wbounds[wb][0]])
                if k % 2 == 0:
                    nc.vector.tensor_scalar(
                        out=osb_s[:, ko:ko + csz], in0=ps2[:, :csz],
                        scalar1=0.0, scalar2=None, op0=mybir.AluOpType.add)
                else:
                    nc.scalar.copy(osb_s[:, ko:ko + csz], ps2[:, :csz])
                if k == wbounds[wb][1] - 1:
                    o0 = int(COFF[wbounds[wb][0]])
                    o1 = int(COFF[wbounds[wb][1]])
                    nc.scalar.dma_start(
                        out_t.ap()[:, o0:o1], osb_s[:, :o1 - o0])
                    wb += 1

    nc.compile()
    return nc


def _probe_check(inputs, out, n_probe=1024, tol=0.35):
    """Exact fp32 recompute of a few output rows to catch transient device
    corruption (first run after boot has been seen returning garbage)."""
    src = np.asarray(inputs["edge_index_sd"][0], dtype=np.int64)
    dst = np.asarray(inputs["edge_index_sd"][1], dtype=np.int64)
    ew = np.asarray(inputs["edge_weight"], dtype=np.float32)
    x_src = np.asarray(inputs["x_src"], dtype=np.float32)
    x_dst = np.asarray(inputs["x_dst"], dtype=np.float32)
    Wn = np.asarray(inputs["W_nei"], dtype=np.float32)
    Ws = np.asarray(inputs["W_self"], dtype=np.float32)
    b = np.asarray(inputs["b_self"], dtype=np.float32)
    rows = np.linspace(0, N_DST - 1, n_probe).astype(np.int64)
    mark = np.zeros(N_DST, dtype=bool)
    mark[rows] = True
    sel = np.flatnonzero(mark[dst])
    agg = np.zeros((N_DST, F), dtype=np.float32)
    np.add.at(agg, dst[sel], x_src[src[sel]] * ew[sel, None])
    exp = agg[rows] @ Wn.T + x_dst[rows] @ Ws.T + b
    return float(np.abs(out[rows] - exp).max()) < tol


def run(inputs, trace=False):
    meta, per_core, common, dstmap = _host_prep(
        inputs["x_src"], inputs["x_dst"], inputs["edge_index_sd"],
        inputs["edge_weight"], inputs["W_nei"], inputs["W_self"],
        inputs["b_self"])
    nc = _build_program(meta)
    in_maps = []
    for c in range(NC):
        m = dict(common)
        m.update(per_core[c])
        in_maps.append(m)
    res = run_bass_kernel_spmd(nc, in_maps, core_ids=list(range(NC)),
                               trace=trace)
    out = np.empty((N_DST, F), dtype=np.float32)
    for c in range(NC):
        out[dstmap[c]] = res.results[c]["outT"].T.astype(np.float32)
    return out, res


def kernel(**inputs) -> np.ndarray:
    # Deep-copy to owned numpy buffers: zero-copy views of jax arrays
    # aliasing into the PJRT execute path break the NEFF compile.
    inputs = {k: np.array(np.asarray(v)) for k, v in inputs.items()}
    last = None
    for attempt in range(3):
        try:
            out, _ = run(inputs, trace=False)
            last = out
        except Exception:
            if attempt == 2:
                raise
            continue
        if _probe_check(inputs, out):
            return out
    return last
